# revision 2
# baseline (speedup 1.0000x reference)
"""Cost-volume kernel for Trainium2 (8 NeuronCores, SPMD).

cost[b,c,h,x,d] = left[b,c,h,x] - right[b,c,h,x-d]  (0 where x < d)
with B,C,H,W = 4,32,128,240 and D = 24.

Sharding: every (b,c,h) row is independent -> flatten to 16384 rows of
W=240, give each of the 8 cores a contiguous 2048-row block (pure data
parallelism, no halo).

The kernel is HBM-write-bound (377 MB output vs 31 MB input), so the
cost volume is computed and stored in bf16 (harness tolerance 2e-2 >>
bf16 rounding ~2e-3) and upcast to f32 on the host: halves store
traffic vs the f32 baseline. Inputs are cast to bf16 and laid out
partition-major on the host so each load DMA is fully contiguous.

Per 128-row tile the [W*D] slab is produced on the DVE only (cross-
engine splits measured slower due to semaphore stalls):
  1. lx8[8w+e] = l[w] materialized by one tensor_copy (broadcast src
     runs ~3.4 elem/cyc; as a tensor_tensor operand a step-0 AP would
     cap the op at 1x, with lx8 all three APs stream step +/-1 at 2x).
  2. rect ops (d-groups of 8, w >= 8g+8): ot[24w+8g+dg] =
     lx8[8w+dg] - r[w-8g-dg]; inner runs of 8, parity-aligned.
  3. corner ops per group cover w in [8g, 8g+8) (56+8 cells).
Invalid cells (x < d, all at slab offset < 552) are zeroed once per
buffer at kernel start and never rewritten.

Stores move [128, 2*5760] bf16 slabs (2.95 MB) on alternating HWDGE
queues (sync/scalar); inputs stream per-chunk through 3 rotating
buffers on gpsimd so loads pipeline behind compute.

Measured per-core HW time ~89-98 us vs 422 us f32 baseline; DMA floor
(25.6 MB @ ~400 GB/s) is ~64 us, DVE busy ~90%.
"""

import sys

if "/opt/trn_rl_repo" not in sys.path:
    sys.path.insert(0, "/opt/trn_rl_repo")

import numpy as np

B, C, H, W, D = 4, 32, 128, 240, 24
P = 128
N_CORES = 8
ROWS = B * C * H                 # 16384
ROWS_PER_CORE = ROWS // N_CORES  # 2048
NT = ROWS_PER_CORE // P          # 16 tiles per core
OTW = W * D                      # 5760
INW = NT * W                     # 3840
CHUNK = 2                        # tiles per store slab
NB = 4                           # out slab buffers
NBIN = 3                         # input chunk buffers

_nc_cache = None


def _build(K=1):
    """Build the per-core Bass kernel; K repeats the whole pipeline
    (K>1 used only by test.py for slope timing)."""
    from concourse import mybir, bacc
    import concourse.tile as tile
    import bass_rust

    bf16 = mybir.dt.bfloat16
    AP = bass_rust.AP
    nc = bacc.Bacc("TRN2", target_bir_lowering=False, debug=False)
    left = nc.dram_tensor("left", [P, INW], bf16, kind="ExternalInput").ap()
    right = nc.dram_tensor("right", [P, INW], bf16, kind="ExternalInput").ap()
    out = nc.dram_tensor("out", [ROWS_PER_CORE, OTW], bf16,
                         kind="ExternalOutput").ap()
    nchunks = NT // CHUNK
    cfree = CHUNK * OTW
    lxw = 8 * W                  # 1920
    with tile.TileContext(nc) as tc:
        with tc.tile_pool(name="p", bufs=1) as pool:
            lts = [pool.tile([P, CHUNK * W], bf16, name=f"lt{i}")
                   for i in range(NBIN)]
            rts = [pool.tile([P, CHUNK * W], bf16, name=f"rt{i}")
                   for i in range(NBIN)]
            ots = [pool.tile([P, cfree], bf16, name=f"ot{i}")
                   for i in range(NB)]
            lxs = [pool.tile([P, CHUNK * lxw], bf16, name=f"lx{i}")
                   for i in range(2)]
            # invalid (x < d) cells all lie in [0, 552) of each tile
            # slab; compute never writes them, so zero once per buffer
            for i in range(NB):
                for j in range(CHUNK):
                    nc.vector.memset(ots[i][:, j * OTW: j * OTW + 552], 0.0)
            for k in range(K):
                for ci in range(nchunks):
                    lt, rt = lts[ci % NBIN], rts[ci % NBIN]
                    sl = slice(ci * CHUNK * W, (ci + 1) * CHUNK * W)
                    nc.gpsimd.dma_start(out=lt[:], in_=left[:, sl])
                    nc.gpsimd.dma_start(out=rt[:], in_=right[:, sl])
                    ot = ots[ci % NB]
                    lx = lxs[ci % 2]
                    otv, ltv, rtv = ot[:].tensor, lt[:].tensor, rt[:].tensor
                    lxv = lx[:].tensor
                    inw = CHUNK * W
                    for j in range(CHUNK):
                        lofs = j * W
                        oofs = j * OTW
                        lxofs = j * lxw
                        # lx8 materialization: lx[8w+e] = l[w]
                        o_ap = AP(tensor=lxv, offset=lxofs,
                                  ap=[[CHUNK * lxw, P], [8, W], [1, 8]])
                        l_ap = AP(tensor=ltv, offset=lofs,
                                  ap=[[inw, P], [1, W], [0, 8]])
                        nc.vector.tensor_copy(out=o_ap, in_=l_ap)
                        for g in range(3):
                            w0 = 8 * g + 8
                            cw = W - w0
                            o_ap = AP(tensor=otv,
                                      offset=oofs + 24 * w0 + 8 * g,
                                      ap=[[cfree, P], [24, cw], [1, 8]])
                            l_ap = AP(tensor=lxv, offset=lxofs + 8 * w0,
                                      ap=[[CHUNK * lxw, P], [8, cw], [1, 8]])
                            r_ap = AP(tensor=rtv, offset=lofs + w0 - 8 * g,
                                      ap=[[inw, P], [1, cw], [-1, 8]])
                            nc.vector.tensor_sub(out=o_ap, in0=l_ap,
                                                 in1=r_ap)
                            # corner: d = 8g+a, w = 8g+a+b, b in [0,8)
                            o2 = AP(tensor=otv, offset=oofs + 200 * g,
                                    ap=[[cfree, P], [25, 8], [24, 8]])
                            l2 = AP(tensor=ltv, offset=lofs + 8 * g,
                                    ap=[[inw, P], [1, 8], [1, 8]])
                            r2 = AP(tensor=rtv, offset=lofs,
                                    ap=[[inw, P], [0, 8], [1, 8]])
                            nc.vector.tensor_sub(out=o2, in0=l2, in1=r2)
                    eng = nc.sync if ci % 2 == 0 else nc.scalar
                    o_ap = AP(tensor=out.tensor,
                              offset=ci * CHUNK * P * OTW,
                              ap=[[OTW, P], [P * OTW, CHUNK], [1, OTW]])
                    s_ap = AP(tensor=otv, offset=0,
                              ap=[[cfree, P], [OTW, CHUNK], [1, OTW]])
                    eng.dma_start(out=o_ap, in_=s_ap)
    nc.compile()
    return nc


def _get_nc():
    global _nc_cache
    if _nc_cache is None:
        _nc_cache = _build()
    return _nc_cache


def _shard_input(full_f32):
    """[B,C,H,W] f32 -> per-core [P, INW] bf16, partition-major, so the
    device load DMA for each chunk is fully contiguous per partition."""
    import ml_dtypes
    flat = np.ascontiguousarray(full_f32, dtype=np.float32).reshape(ROWS, W)
    bf = flat.astype(ml_dtypes.bfloat16)
    per = bf.reshape(N_CORES, NT, P, W).transpose(0, 2, 1, 3).reshape(
        N_CORES, P, INW)
    return np.ascontiguousarray(per)


def kernel(left_img: np.ndarray, right_img: np.ndarray) -> np.ndarray:
    from concourse.bass_utils import run_bass_kernel_spmd

    nc = _get_nc()
    ls = _shard_input(left_img)
    rs = _shard_input(right_img)
    in_maps = [{"left": ls[i], "right": rs[i]} for i in range(N_CORES)]
    res = run_bass_kernel_spmd(nc, in_maps, list(range(N_CORES)))
    shards = [res.results[i]["out"] for i in range(N_CORES)]
    full = np.concatenate(shards, axis=0).astype(np.float32)
    return full.reshape(B, C, H, W, D)
